# revision 8
# baseline (speedup 1.0000x reference)
"""Trainium2 Bass kernel for nn_AnalysisModel (8 NeuronCores, batch-parallel).

Distribution: data-parallel over batch — core c computes batch element c
end-to-end (B == n_cores == 8). No collectives.

Structure (v2, pipelined):
 - Collapsed recurrence u_t = sin(sqrt2*rinv_t*u_{t-1} + 2*b_t + 2*t*PHI
   + pi/4) runs chunk-parallel (64 chunks of 8 steps, 64-step warmup) as a
   SINGLE group entirely on the vector engine: add + fused mult-range-wrap
   + degree-7 odd polynomial sin (custom DVE ops) — no cross-engine syncs.
 - Attention is triangular: scores for key-block si only cover queries
   >= 128*si; 4 heads run as concurrent PE row-groups. ctx+den share one
   matmul via a ones-column. Work is emitted per 128-query block tb so
   logits matmuls + PSUM->SBUF copies + output DMA for tb=0 overlap the
   attention/resonant compute of later blocks.
 - Resonant layer: order-4 Taylor around alpha = x + t*PHI (18 rank-64
   matmuls per block); sin/cos of alpha via the DVE poly; silu via
   exp(-y) + reciprocal so the scalar engine's ACT table never leaves Exp.
 - Logits stream per (tb, 8-pair group): 2MB staged writes on the sync
   queue; input DMAs ride the scalar/gpsimd queues so nothing serializes
   behind the 4MB w_out stripe.
"""

import math

import numpy as np

import concourse.bass as bass
import concourse.mybir as mybir
import concourse.tile as tile
from concourse import bacc
from concourse.bass_utils import run_bass_kernel_spmd
from concourse.masks import make_identity

import concourse.dve_ops as _dvo
from concourse.dve_spec import (
    Spec as _Spec, Src0 as _Src0, Src1 as _Src1,
    C0 as _C0, C1 as _C1, C2 as _C2, C3 as _C3,
    lower as _dve_lower, _spill_c3_to_src1,
)
from concourse.dve_uop import DveOpSpec as _DveOpSpec

# sin(x) ~= x*(S1 + z*(S3 + z*(S5 + z*S7))), z = x^2, minimax on [-pi, pi]
SC1, SC3, SC5, SC7 = (9.99277317e-01, -1.65668376e-01,
                      7.95838010e-03, -1.45097151e-04)


def _register_op(name, spec):
    for op in _dvo.OPS:
        if op.name == name:
            return op
    row = _dvo._CUSTOM_DVE_ROW_BASE + len(_dvo.OPS)
    _dvo._SUB_OPCODE_FOR_NAME[name] = row
    shas = {}
    for ver in ("v3",):
        uops = _dve_lower(spec, ver=ver)
        tmp = _DveOpSpec(name=name, opcode=row, uops=uops, rd1_en=True)
        shas[ver] = tmp.sha(ver)
    op = _dvo.DveOp(name, spec, subdim=False, uops_sha=shas)
    _dvo.OPS.append(op)
    _dvo.CUSTOM_DVE_SPECS[name] = spec
    return op


def _register_mult_range_wrap():
    """out = y + imm2*((y < -s1) - (y > s1)) with y = in0*in1."""
    _y = _Src0 * _Src1
    def _ref(in0, in1, s0, s1, imm2):
        y = (in0.astype(np.float32) * in1).astype(np.float32)
        return y + imm2 * (
            (y < -s1).astype(np.float32) - (y > s1).astype(np.float32)
        )
    return _register_op(
        "MULT_RANGE_WRAP_ANT",
        _Spec(body=_y + _C2 * ((_y < -_C1) - (_y > _C1)), reference=_ref),
    )


def _register_sinpoly():
    """out = in0*(s0 + z*(s1 + z*(imm2 + z*c7))), z = in0^2; c7 rides the
    C3->Src1 spill (in1 = [P,1] tile holding SC7). Valid on [-pi, pi]."""
    _z = _Src0 * _Src0
    _h = ((_C3 * _z + _C2) * _z + _C1) * _z + _C0
    body = _spill_c3_to_src1(_h * _Src0)
    def _ref(in0, in1, s0, s1, imm2):
        x = in0.astype(np.float32)
        z = x * x
        c7 = np.asarray(in1, dtype=np.float32)
        if c7.ndim and c7.shape[-1] == 1:
            pass  # broadcast [P,1]
        return (x * (s0 + z * (s1 + z * (imm2 + z * c7)))).astype(np.float32)
    return _register_op("SINPOLY7_ANT", _Spec(body=body, reference=_ref))


_MRW = _register_mult_range_wrap()
_SINP = _register_sinpoly()

F32 = mybir.dt.float32
I16 = mybir.dt.int16
F16 = mybir.dt.float16
AF = mybir.ActivationFunctionType
OP = mybir.AluOpType

B, S, V, D, H, DH, N = 8, 512, 32000, 64, 4, 16, 128
PHI = 1.618033988749895
PI = float(np.pi)
SQRT2 = float(np.sqrt(2.0))

W_WARM = 64          # warmup steps for the chunk-parallel recurrence
CS = 8               # chunk size (time steps per chunk)
NCH = S // CS        # 64 chunks
L_REC = W_WARM + CS

NPAIR_TOT = 31       # full 1024-wide pairs in the packed w_out
WPW = 16128          # packed w_out width (31*512 + 256 tail)
# pack64 layout (f32): wq bq wk bk | wres[128] bres[128] | tp1[512] tp2[512]
PK_WQ, PK_BQ, PK_WK, PK_BK = 0, 1, 2, 3
PK_WRES, PK_BRES, PK_TP1, PK_TP2, PK_END = 4, 132, 260, 772, 1284


def _sinpoly(nc, out_ap, in_ap, c7t):
    nc.vector._custom_dve(
        _SINP, out=out_ap, in0=in_ap, in1=c7t,
        s0=SC1, s1=SC3, imm2=SC5,
    )


def build_nc():
    nc = bacc.Bacc("TRN2", target_bir_lowering=False)

    ids16 = nc.dram_tensor("ids16", [128, 32], I16, kind="ExternalInput")
    emb = nc.dram_tensor("emb", [V, 2 * D], F32, kind="ExternalInput")
    pack64 = nc.dram_tensor("pack64", [64, PK_END], F32, kind="ExternalInput")
    pack128 = nc.dram_tensor("pack128", [128, 192], F16, kind="ExternalInput")
    wout_t = nc.dram_tensor("wout_t", [128, WPW], F16, kind="ExternalInput")
    out = nc.dram_tensor("out", [S, V], F16, kind="ExternalOutput")

    with tile.TileContext(nc) as tc:
        with (
            tc.tile_pool(name="const", bufs=1) as cp,
            tc.tile_pool(name="work", bufs=2) as wp,
            tc.tile_pool(name="bcoef", bufs=1) as bp,
            tc.tile_pool(name="rdp", bufs=2) as rp,
            tc.tile_pool(name="stage", bufs=3) as lsb,
        ):
            # ================ phase 0: loads ================
            # gpsimd queue: ids + the two gathers (SWDGE). sync queue: the
            # 4MB w_out stripe (in flight during the whole recurrence).
            # scalar queue: packed weights (HWDGE, doesn't queue behind
            # w_out).
            ids_sb = cp.tile([128, 32], I16)
            nc.gpsimd.dma_start(ids_sb[:], ids16[:])
            wpair = cp.tile([128, WPW], F16)
            nc.sync.dma_start(wpair[:, 0:WPW // 2], wout_t[:, 0:WPW // 2])
            nc.sync.dma_start(wpair[:, WPW // 2:], wout_t[:, WPW // 2:])
            pk64 = cp.tile([64, PK_END], F32)
            nc.scalar.dma_start(pk64[:], pack64[:])
            pk128 = cp.tile([128, 192], F16)
            nc.scalar.dma_start(pk128[:], pack128[:])

            e_raw = cp.tile([128, 2, 128], F32)
            nc.gpsimd.dma_gather(
                e_raw[:], emb[:], ids_sb[:, 0:16], num_idxs=S // 2,
                num_idxs_reg=S // 2, elem_size=2 * D,
            )
            e_raw2 = cp.tile([128, 2, 128], F32)
            nc.gpsimd.dma_gather(
                e_raw2[:], emb[:], ids_sb[:, 16:32], num_idxs=S // 2,
                num_idxs_reg=S // 2, elem_size=2 * D,
            )

            wq_sb = pk64[:, PK_WQ:PK_WQ + 1]
            bq_sb = pk64[:, PK_BQ:PK_BQ + 1]
            wk_sb = pk64[:, PK_WK:PK_WK + 1]
            bk_sb = pk64[:, PK_BK:PK_BK + 1]
            wres_sb = pk64[:, PK_WRES:PK_WRES + N]
            beta = pk64[:, PK_BRES:PK_BRES + N]
            tp1_64 = pk64[:, PK_TP1:PK_TP1 + S]
            tp2_64 = pk64[:, PK_TP2:PK_TP2 + S]
            wpr_sb = pk128[:, 0:64]
            wpi_sb = pk128[:, 64:128]
            wctx_sb = pk128[0:64, 128:192]

            ident = cp.tile([128, 128], F32)
            make_identity(nc, ident[:])

            # ---- gpsimd-built constants (PE/vector stay free) ----
            # head-duplication matrix: P4[d, p] = 1 iff d == 16*(p//32)+p%16
            pmat4 = cp.tile([64, 128], F32)
            nc.gpsimd.memset(pmat4[:], 0.0)
            nc.gpsimd.affine_select(
                out=pmat4[:], in_=pmat4[:], compare_op=OP.not_equal, fill=1.0,
                base=0, channel_multiplier=1,
                pattern=[[-16, 4], [0, 2], [-1, 16]],
            )
            halfq4 = cp.tile([128, 1], F32)
            nc.gpsimd.memset(halfq4[:], 0.0)
            for hb in range(4):
                nc.gpsimd.memset(halfq4[32 * hb:32 * hb + 16, :], PI / 2)
            # triangular keep-mask T01[p, j] = 1 iff p < j
            t01f = wp.tile([128, 128], F32, tag="t01f")
            nc.gpsimd.memset(t01f[:], 1.0)
            nc.gpsimd.affine_select(
                out=t01f[:], in_=t01f[:], compare_op=OP.is_ge, fill=0.0,
                base=-1, channel_multiplier=-1, pattern=[[1, 128]],
            )
            t01 = cp.tile([128, 128], F16)
            nc.gpsimd.tensor_copy(t01[:], t01f[:])
            dup128f = wp.tile([64, 128], F32, tag="dup128f")
            nc.gpsimd.memset(dup128f[:], 0.0)
            nc.gpsimd.affine_select(
                out=dup128f[:], in_=dup128f[:], compare_op=OP.not_equal,
                fill=1.0, base=0, channel_multiplier=1,
                pattern=[[0, 2], [-1, 64]],
            )
            dup128 = cp.tile([64, 128], F16)
            nc.gpsimd.tensor_copy(dup128[:], dup128f[:])
            ones128 = cp.tile([128, 1], F32)
            nc.gpsimd.memset(ones128[:], 1.0)
            c7t = cp.tile([64, 1], F32)
            nc.gpsimd.memset(c7t[:], SC7)
            c7t128 = cp.tile([128, 1], F32)
            nc.gpsimd.memset(c7t128[:], SC7)

            # ---- resonant Taylor coefficient tables (gpsimd + 2 recips) ----
            aresw = bp.tile([64, N], F32, tag="aresw")
            nc.vector.scalar_tensor_tensor(aresw[:], wres_sb, -1.0, wres_sb, OP.mult, OP.max)
            nc.vector.tensor_scalar(aresw[:], aresw[:], 1.0, 0.0, OP.add, OP.add)
            rres = bp.tile([64, N], F32, tag="rres")
            rscr = bp.tile([64, N], F32, tag="rscr")
            nc.vector.reciprocal_approx_accurate(rres[:], aresw[:], rscr[:])
            rho = bp.tile([64, N], F32, tag="rho")
            nc.vector.tensor_scalar(rho[:], rres[:], -1.0, 0.0, OP.add, OP.add)

            _uid = [0]
            def tmul(x_, y_, tag):
                _uid[0] += 1
                t = bp.tile([64, N], F32, tag=f"bt{_uid[0]}_{tag}")
                nc.gpsimd.tensor_mul(t[:], x_, y_)
                return t

            rho2 = tmul(rho[:], rho[:], "rho2")
            rho3 = tmul(rho2[:], rho[:], "rho3")
            rho4 = tmul(rho2[:], rho2[:], "rho4")
            b2 = tmul(beta, beta, "b2")
            b3 = tmul(b2[:], beta, "b3")
            b4 = tmul(b2[:], b2[:], "b4")

            def combo(tag, terms, const=None):
                _uid[0] += 1
                acc = bp.tile([64, N], F32, tag=f"bc{_uid[0]}_{tag}")
                first = True
                for cf, t_ in terms:
                    if first:
                        nc.vector.tensor_scalar(acc[:], t_, cf, 0.0, OP.mult, OP.add)
                        first = False
                    else:
                        nc.vector.scalar_tensor_tensor(acc[:], t_, cf, acc[:], OP.mult, OP.add)
                if const is not None:
                    nc.vector.tensor_scalar(acc[:], acc[:], const, 0.0, OP.add, OP.add)
                return acc

            cc = {}
            cs = {}
            cc[0] = combo("cc0", [(-0.5, b2[:]), (1.0 / 24, b4[:])], const=1.0)
            cc1t = combo("cc1t", [(-1.0, beta), (1.0 / 6, b3[:])])
            cc[1] = tmul(cc1t[:], rho[:], "w64n")
            cc2t = combo("cc2t", [(0.25, b2[:])], const=-0.5)
            cc[2] = tmul(cc2t[:], rho2[:], "w64n")
            cc3t = tmul(beta, rho3[:], "w64n")
            cc[3] = combo("cc3", [(1.0 / 6, cc3t[:])])
            cc[4] = combo("cc4", [(1.0 / 24, rho4[:])])
            cs[0] = combo("cs0", [(1.0, beta), (-1.0 / 6, b3[:])])
            cs1t = combo("cs1t", [(-0.5, b2[:])], const=1.0)
            cs[1] = tmul(cs1t[:], rho[:], "w64n")
            cs2t = tmul(beta, rho2[:], "w64n")
            cs[2] = combo("cs2", [(-0.5, cs2t[:])])
            cs[3] = combo("cs3", [(-1.0 / 6, rho3[:])])

            cc_r, csp_r, csn_r = {}, {}, {}
            for jx in range(5):
                t_ = cp.tile([64, N], F16, tag=f"ccr{jx}")
                nc.gpsimd.tensor_copy(t_[:], cc[jx][:])
                cc_r[jx] = t_
            for jx in range(4):
                t_ = cp.tile([64, N], F16, tag=f"cspr{jx}")
                nc.gpsimd.tensor_copy(t_[:], cs[jx][:])
                csp_r[jx] = t_
                t2_ = cp.tile([64, N], F16, tag=f"csnr{jx}")
                nc.vector.tensor_scalar(t2_[:], cs[jx][:], -1.0, 0.0, OP.mult, OP.add)
                csn_r[jx] = t2_

            # q/k per-head reciprocal scales (vector; tiny)
            rq = cp.tile([64, 1], F32)
            rk = cp.tile([64, 1], F32)
            t64a = bp.tile([64, 1], F32, tag="t64a")
            t64s = bp.tile([64, 1], F32, tag="t64s")
            nc.vector.scalar_tensor_tensor(t64a[:], wq_sb, -1.0, wq_sb, OP.mult, OP.max)
            nc.vector.tensor_scalar(t64a[:], t64a[:], 1.0, 0.0, OP.add, OP.add)
            nc.vector.reciprocal_approx_accurate(rq[:], t64a[:], t64s[:])
            t64b = bp.tile([64, 1], F32, tag="t64b")
            t64u = bp.tile([64, 1], F32, tag="t64u")
            nc.vector.scalar_tensor_tensor(t64b[:], wk_sb, -1.0, wk_sb, OP.mult, OP.max)
            nc.vector.tensor_scalar(t64b[:], t64b[:], 1.0, 0.0, OP.add, OP.add)
            nc.vector.reciprocal_approx_accurate(rk[:], t64b[:], t64u[:])

            # ---- long-lived state tiles ----
            states = cp.tile([64, S], F32)
            states_s = cp.tile([64, S], F32)
            s_arr = cp.tile([64, S], F32)
            r_arr = cp.tile([64, S], F32)
            q4 = cp.tile([128, S], F16)
            k4 = cp.tile([128, S], F16)
            ctx_sum = cp.tile([64, S], F32)
            x_t = cp.tile([64, S], F32)

            # ================ emb transpose + recurrence params ================
            with tc.tile_pool(name="psPre", bufs=2, space="PSUM") as psPre:
                w_embT = cp.tile([64, S], F32)
                b_embT = cp.tile([64, S], F32)
                for c in range(4):
                    esrc = e_raw if c < 2 else e_raw2
                    cc_ = c % 2
                    tpw = psPre.tile([64, 128], F32, tag="tp")
                    nc.tensor.transpose(tpw[:], esrc[:, cc_, 0:64], ident[:])
                    nc.vector.tensor_copy(w_embT[:, c * 128:(c + 1) * 128], tpw[:])
                    tpb = psPre.tile([64, 128], F32, tag="tp")
                    nc.tensor.transpose(tpb[:], esrc[:, cc_, 64:128], ident[:])
                    nc.vector.tensor_copy(b_embT[:, c * 128:(c + 1) * 128], tpb[:])
                w_emb = w_embT[:, :]
                b_emb = b_embT[:, :]

                awt = wp.tile([64, S], F32, tag="wa")
                nc.vector.scalar_tensor_tensor(awt[:], w_emb, -1.0, w_emb, OP.mult, OP.max)
                wl = wp.tile([64, S], F32, tag="wwl")
                nc.vector.tensor_scalar(wl[:], awt[:], 1.0, 0.0, OP.add, OP.add)
                rinv = wp.tile([64, S], F32, tag="wri")
                scr = wp.tile([64, S], F32, tag="wsc")
                nc.vector.reciprocal_approx_accurate(rinv[:], wl[:], scr[:])
                nc.vector.tensor_scalar(s_arr[:], rinv[:], SQRT2, 0.0, OP.mult, OP.add)
                bh = wp.tile([64, S], F32, tag="wbh")
                nc.vector.scalar_tensor_tensor(bh[:], b_emb, 2.0, tp2_64, OP.mult, OP.add)
                bwr = wp.tile([64, S], F32, tag="wbw")
                nc.vector.add_range_wrap(bwr[:], bh[:], 0.0, PI, 2 * PI)
                r_tmp = wp.tile([64, S], F32, tag="wrt")
                nc.gpsimd.tensor_mul(r_tmp[:], bwr[:], wl[:])
                nc.vector.tensor_scalar(r_arr[:], r_tmp[:], 1.0 / SQRT2, 0.0, OP.mult, OP.add)

                # q/k angle precursors that don't need states
                tpq = cp.tile([64, S], F32)
                nc.vector.tensor_scalar(tpq[:], tp1_64, bq_sb, 0.0, OP.add, OP.add)

            # ================ phase 1: chunked recurrence (vector-only) ======
            s3 = s_arr[:].rearrange("d (c s) -> d c s", s=CS)
            r3 = r_arr[:].rearrange("d (c s) -> d c s", s=CS)
            st3 = states[:].rearrange("d (c s) -> d c s", s=CS)

            with tc.tile_pool(name="psWarm", bufs=1, space="PSUM") as psWarm:
                u_sg = cp.tile([64, NCH], F32)
                nc.vector.memset(u_sg[:], 0.0)
                v_sg = cp.tile([64, NCH], F32)
                tw_sg = cp.tile([64, NCH], F32)

                for j in range(L_REC):
                    jj = j - W_WARM
                    r8 = jj % CS
                    c0 = max(0, math.ceil(-jj / CS))
                    nf = NCH - c0
                    s0g = c0 + (jj - r8) // CS
                    s_sl = s3[:, s0g:s0g + nf, r8]
                    r_sl = r3[:, s0g:s0g + nf, r8]
                    if jj >= 1:
                        u_rd = st3[:, c0:NCH, jj - 1]
                    else:
                        u_rd = u_sg[:, c0:]
                    nc.vector.tensor_add(v_sg[:, c0:], u_rd, r_sl)
                    nc.vector._custom_dve(
                        _MRW, out=tw_sg[:, c0:], in0=v_sg[:, c0:],
                        in1=s_sl, s0=0.0, s1=PI, imm2=2 * PI,
                    )
                    if jj >= 0:
                        w_ap = st3[:, c0:NCH, jj]
                    else:
                        w_ap = u_sg[:, c0:]
                    _sinpoly(nc, w_ap, tw_sg[:, c0:], c7t[:])
                    # PE warm-keeper: a throwaway matmul reading v_sg keeps
                    # the HAM activity window non-idle so post-recurrence
                    # matmuls start at 2.4 GHz.
                    if j % 4 == 3:
                        wps = psWarm.tile([64, 128], F32, tag="warm")
                        nc.tensor.matmul(
                            wps[:], v_sg[:], ident[0:64, :],
                            start=True, stop=True,
                        )

            nc.vector.tensor_scalar(states_s[:], states[:], SQRT2, 0.0, OP.mult, OP.add)

            # ================ phase 2a: q/k build ================
            with tc.tile_pool(name="psMid", bufs=2, space="PSUM") as psMid:
                thq = wp.tile([64, S], F32, tag="w64")
                nc.vector.scalar_tensor_tensor(thq[:], states_s[:], rq[:], tpq[:], OP.mult, OP.add)
                thqw = wp.tile([64, S], F32, tag="w64b")
                nc.vector.add_range_wrap(thqw[:], thq[:], 0.0, PI, 2 * PI)
                bkb = bk_sb.broadcast_to((64, S))
                thk = wp.tile([64, S], F32, tag="w64")
                nc.vector.scalar_tensor_tensor(thk[:], states_s[:], rk[:], bkb, OP.mult, OP.add)
                thkw = wp.tile([64, S], F32, tag="w64b")
                nc.vector.add_range_wrap(thkw[:], thk[:], 0.0, PI, 2 * PI)

                for src_, dst in ((thqw, q4), (thkw, k4)):
                    dup = psMid.tile([128, S], F32, tag="dup")
                    nc.tensor.matmul(dup[:], pmat4[:], src_[:])
                    dwr = wp.tile([128, S], F32, tag="w128")
                    nc.vector.add_range_wrap(dwr[:], dup[:], halfq4[:], PI, 2 * PI)
                    _sinpoly(nc, dst[:], dwr[:], c7t128[:])

                # statesT (+ ones column) for the ctx/den matmuls
                stT = []
                for si in range(4):
                    tp = psMid.tile([128, 128], F32, tag="tp2")
                    nc.tensor.transpose(
                        tp[:, 0:64], states_s[:, 128 * si:128 * (si + 1)],
                        ident[0:64, 0:64],
                    )
                    t_ = cp.tile([128, 65], F16, tag=f"stT{si}")
                    nc.vector.tensor_copy(t_[:, 0:64], tp[:, 0:64])
                    nc.vector.tensor_copy(t_[:, 64:65], ones128[:])
                    stT.append(t_)

            # ================ phases 2b/3/4: per-query-block pipeline ========
            inv_scale = 1.0 / float(np.sqrt(2.0 * DH))
            with (
                tc.tile_pool(name="psS", bufs=2, space="PSUM") as psS,
                tc.tile_pool(name="psC", bufs=1, space="PSUM") as psC,
                tc.tile_pool(name="psR", bufs=1, space="PSUM") as psR,
                tc.tile_pool(name="psL", bufs=2, space="PSUM") as psL,
            ):
                ex = {}
                copy_ctr = [0]

                def psum_to_sbuf(dst_ap, src_ap):
                    if copy_ctr[0] % 2 == 0:
                        nc.vector.tensor_copy(dst_ap, src_ap)
                    else:
                        nc.scalar.copy(dst_ap, src_ap)
                    copy_ctr[0] += 1

                for tb in range(4):
                    si = tb
                    ncols = S - 128 * si
                    # -------- scores + exp + diagonal mask for key-block si
                    for h in range(4):
                        p0 = 32 * h
                        sc = psS.tile([128, 512], F32, tag="sc")
                        nc.tensor.matmul(
                            sc[:, :ncols],
                            k4[p0:p0 + 32, 128 * si:128 * (si + 1)],
                            q4[p0:p0 + 32, 128 * si:],
                            start=True, stop=True, tile_position=(p0, 0),
                        )
                        e_ = cp.tile([128, ncols], F16, tag=f"ex{si}_{h}")
                        nc.scalar.activation(
                            e_[:, :ncols], sc[:, :ncols], AF.Exp,
                            scale=inv_scale,
                        )
                        # diagonal block: keep keys strictly before query
                        nc.gpsimd.tensor_mul(
                            e_[:, 0:128], e_[:, 0:128], t01[:],
                        )
                        ex[(si, h)] = e_

                    # -------- ctx+den for query block tb (keys si' <= tb)
                    ctx4 = psC.tile([65, 512], F32, tag="ctx4")
                    for h in range(4):
                        for sp in range(tb + 1):
                            lo = (tb - sp) * 128
                            nc.tensor.matmul(
                                ctx4[:, 128 * h:128 * (h + 1)],
                                stT[sp][:, 0:65],
                                ex[(sp, h)][:, lo:lo + 128],
                                start=(sp == 0), stop=(sp == tb),
                            )
                    tbsl = slice(128 * tb, 128 * (tb + 1))
                    for h in range(4):
                        rd0 = rp.tile([1, 128], F32, tag="rd0")
                        rds_ = rp.tile([1, 128], F32, tag="rds")
                        nc.vector.reciprocal_approx_accurate(
                            rd0[:], ctx4[64:65, 128 * h:128 * (h + 1)], rds_[:]
                        )
                        if tb == 0:
                            nc.vector.memset(rd0[0:1, 0:1], 0.0)
                        rdb = wp.tile([64, 128], F32, tag="rdb")
                        nc.gpsimd.partition_broadcast(rdb[:], rd0[:])
                        if h == 0:
                            nc.vector.tensor_mul(
                                ctx_sum[:, tbsl], ctx4[0:64, 0:128], rdb[:]
                            )
                        else:
                            csh = wp.tile([64, 128], F32, tag="csh")
                            nc.vector.tensor_mul(
                                csh[:], ctx4[0:64, 128 * h:128 * (h + 1)], rdb[:]
                            )
                            nc.vector.tensor_add(
                                ctx_sum[:, tbsl], ctx_sum[:, tbsl], csh[:]
                            )
                    ctx16 = wp.tile([64, 128], F16, tag="ctx16")
                    nc.vector.tensor_copy(ctx16[:], ctx_sum[:, tbsl])
                    # wctx matmul reuses the consumed ctx4 bank
                    cp_ps = ctx4[0:64, 384:512]
                    nc.tensor.matmul(cp_ps, wctx_sb, ctx16[:], start=True, stop=True)
                    nc.vector.tensor_add(x_t[:, tbsl], states_s[:, tbsl], cp_ps)

                    # -------- resonant layer for tb (order-4 Taylor)
                    resp = psR.tile([128, 512], F32, tag="resp")
                    cos_ps = resp[:, 0:128]
                    sin_ps = resp[:, 128:256]
                    y_ps = resp[0:64, 256:384]
                    xps = resp[:, 384:512]

                    alpha = wp.tile([64, 128], F32, tag="ra")
                    nc.gpsimd.tensor_add(alpha[:], x_t[:, tbsl], tp1_64[:, tbsl])
                    aw = wp.tile([64, 128], F32, tag="raw")
                    nc.vector.add_range_wrap(aw[:], alpha[:], 0.0, PI, 2 * PI)
                    ac_in = wp.tile([64, 128], F32, tag="rac")
                    nc.vector.add_range_wrap(ac_in[:], aw[:], PI / 2, PI, 2 * PI)
                    sa_f = wp.tile([64, 128], F32, tag="rsa")
                    _sinpoly(nc, sa_f[:], aw[:], c7t[:])
                    ca_f = wp.tile([64, 128], F32, tag="rca")
                    _sinpoly(nc, ca_f[:], ac_in[:], c7t[:])
                    x2 = wp.tile([64, 128], F32, tag="rx2")
                    nc.gpsimd.tensor_mul(x2[:], x_t[:, tbsl], x_t[:, tbsl])
                    x3 = wp.tile([64, 128], F32, tag="rx3")
                    nc.gpsimd.tensor_mul(x3[:], x2[:], x_t[:, tbsl])
                    x4 = wp.tile([64, 128], F32, tag="rx4")
                    nc.gpsimd.tensor_mul(x4[:], x2[:], x2[:])
                    xp_ = {1: x_t[:, tbsl], 2: x2[:], 3: x3[:], 4: x4[:]}
                    a_c = {}
                    a_s = {}
                    t_ = wp.tile([64, 128], F16, tag="rac0")
                    nc.gpsimd.tensor_copy(t_[:], ca_f[:])
                    a_c[0] = t_
                    t_ = wp.tile([64, 128], F16, tag="ras0")
                    nc.gpsimd.tensor_copy(t_[:], sa_f[:])
                    a_s[0] = t_
                    for a in range(1, 5):
                        tc_ = wp.tile([64, 128], F16, tag=f"racm{a}")
                        nc.gpsimd.tensor_mul(tc_[:], xp_[a], ca_f[:])
                        a_c[a] = tc_
                        ts_ = wp.tile([64, 128], F16, tag=f"rasm{a}")
                        nc.gpsimd.tensor_mul(ts_[:], xp_[a], sa_f[:])
                        a_s[a] = ts_

                    for jx in range(5):
                        nc.tensor.matmul(cos_ps, cc_r[jx][:], a_c[jx][:],
                                         start=(jx == 0), stop=False)
                    for jx in range(4):
                        nc.tensor.matmul(cos_ps, csn_r[jx][:], a_s[jx][:],
                                         start=False, stop=(jx == 3))
                    for jx in range(5):
                        nc.tensor.matmul(sin_ps, cc_r[jx][:], a_s[jx][:],
                                         start=(jx == 0), stop=False)
                    for jx in range(4):
                        nc.tensor.matmul(sin_ps, csp_r[jx][:], a_c[jx][:],
                                         start=False, stop=(jx == 3))
                    cos16 = wp.tile([128, 128], F16, tag="rc16")
                    nc.vector.tensor_copy(cos16[:], cos_ps)
                    sin16 = wp.tile([128, 128], F16, tag="rs16")
                    nc.vector.tensor_copy(sin16[:], sin_ps)
                    nc.tensor.matmul(y_ps, wpr_sb, cos16[:], start=True, stop=False)
                    nc.tensor.matmul(y_ps, wpi_sb, sin16[:], start=False, stop=True)
                    # silu(y) = y / (1 + exp(-y)) — scalar stays on the Exp table
                    eneg = wp.tile([64, 128], F32, tag="ren")
                    nc.scalar.activation(eneg[:], y_ps, AF.Exp, scale=-1.0)
                    d1 = wp.tile([64, 128], F32, tag="rd1")
                    nc.vector.tensor_scalar(d1[:], eneg[:], 1.0, 0.0, OP.add, OP.add)
                    rsig = wp.tile([64, 128], F32, tag="rsg")
                    rssc = wp.tile([64, 128], F32, tag="rsc")
                    nc.vector.reciprocal_approx_accurate(rsig[:], d1[:], rssc[:])
                    y_sb = wp.tile([64, 128], F32, tag="rys")
                    nc.vector.tensor_copy(y_sb[:], y_ps)
                    sil = wp.tile([64, 128], F32, tag="rsil")
                    nc.gpsimd.tensor_mul(sil[:], y_sb[:], rsig[:])
                    xf16 = wp.tile([64, 128], F16, tag="rxf")
                    nc.gpsimd.tensor_add(xf16[:], x_t[:, tbsl], sil[:])
                    nc.tensor.matmul(xps, dup128[:], xf16[:], start=True, stop=True)
                    xf2 = wp.tile([128, 128], F16, tag="rxf2")
                    nc.vector.tensor_copy(xf2[:], xps)

                    # -------- logits + writeback for tb
                    for g in range(4):
                        p_lo = 8 * g
                        p_hi = min(8 * (g + 1), NPAIR_TOT)
                        gw = (p_hi - p_lo) * 1024 + (256 if g == 3 else 0)
                        st = lsb.tile([128, 8192], F16, tag="st")
                        for p_ in range(p_lo, p_hi):
                            pc = 512 * p_
                            h0 = (p_ - p_lo) * 1024
                            pt = psL.tile([128, 1024], F32, tag="pt")
                            nc.tensor.matmul(
                                pt[:, 0:512], xf2[0:64, :],
                                wpair[0:64, pc:pc + 512], start=True, stop=True,
                                tile_position=(0, 0),
                            )
                            nc.tensor.matmul(
                                pt[:, 512:1024], xf2[64:128, :],
                                wpair[64:128, pc:pc + 512], start=True, stop=True,
                                tile_position=(64, 0),
                            )
                            psum_to_sbuf(st[:, h0:h0 + 1024], pt[:])
                        if g == 3:
                            h0 = (p_hi - p_lo) * 1024
                            pt = psL.tile([128, 1024], F32, tag="pt")
                            nc.tensor.matmul(
                                pt[:, 0:256], xf2[0:64, :],
                                wpair[0:64, 15872:16128],
                                start=True, stop=True, tile_position=(0, 0),
                            )
                            psum_to_sbuf(st[:, h0:h0 + 256], pt[:, 0:256])
                        v0 = 8192 * g
                        nc.sync.dma_start(
                            out[128 * tb:128 * (tb + 1), v0:v0 + gw],
                            st[:, :gw],
                        )

    nc.compile()
    return nc


_NC_CACHE = None


def _host_inputs(inputs):
    """Build the per-core DRAM input maps from the full model inputs."""
    ids = np.asarray(inputs["input_ids"]).astype(np.int64)
    emb_in = np.ascontiguousarray(np.asarray(inputs["emb"], dtype=np.float32))

    pack = np.zeros((64, PK_END), np.float32)
    pack[:, PK_WQ] = np.asarray(inputs["w_query"], np.float32).reshape(64)
    pack[:, PK_BQ] = np.asarray(inputs["b_query"], np.float32).reshape(64)
    pack[:, PK_WK] = np.asarray(inputs["w_key"], np.float32).reshape(64)
    pack[:, PK_BK] = np.asarray(inputs["b_key"], np.float32).reshape(64)
    pack[:, PK_WRES:PK_WRES + N] = np.asarray(inputs["W_res"], np.float32).T
    pack[:, PK_BRES:PK_BRES + N] = np.asarray(inputs["B_res"], np.float32).T

    t64 = np.arange(S, dtype=np.float64)
    tp1 = np.mod(PHI * t64, 2 * np.pi)
    tp1[tp1 >= np.pi] -= 2 * np.pi
    tp2 = np.mod(2 * PHI * t64 + np.pi / 4, 2 * np.pi)
    tp2[tp2 >= np.pi] -= 2 * np.pi
    pack[:, PK_TP1:PK_TP1 + S] = tp1.astype(np.float32)[None, :]
    pack[:, PK_TP2:PK_TP2 + S] = tp2.astype(np.float32)[None, :]
    pack = np.ascontiguousarray(pack)

    p128 = np.zeros((128, 192), np.float16)
    p128[:, 0:64] = np.asarray(inputs["w_pr"], np.float32).T.astype(np.float16)
    p128[:, 64:128] = np.asarray(inputs["w_pi"], np.float32).T.astype(np.float16)
    p128[0:64, 128:192] = np.asarray(inputs["w_ctx"], np.float32).T.astype(np.float16)
    p128 = np.ascontiguousarray(p128)

    wout_T = np.asarray(inputs["w_out"], dtype=np.float32).T  # [64, V]
    wT16 = wout_T.astype(np.float16)
    wout_pack = np.zeros((128, WPW), np.float16)
    for p in range(NPAIR_TOT):
        wout_pack[0:64, 512 * p:512 * (p + 1)] = wT16[:, 1024 * p:1024 * p + 512]
        wout_pack[64:128, 512 * p:512 * (p + 1)] = wT16[:, 1024 * p + 512:1024 * (p + 1)]
    wout_pack[0:64, 15872:16128] = wT16[:, 31744:32000]

    common = dict(emb=emb_in, pack64=pack, pack128=p128, wout_t=wout_pack)
    in_maps = []
    for b in range(B):
        ids16 = np.zeros((128, 32), np.int16)
        for i in range(S):
            ids16[i % 16, i // 16] = ids[b, i]
        ids16 = np.ascontiguousarray(np.tile(ids16[0:16], (8, 1)))
        m = dict(common)
        m["ids16"] = ids16
        in_maps.append(m)
    return in_maps


def kernel(**inputs):
    global _NC_CACHE
    if _NC_CACHE is None:
        _NC_CACHE = build_nc()
    nc = _NC_CACHE
    in_maps = _host_inputs(inputs)
    res = run_bass_kernel_spmd(nc, in_maps, core_ids=list(range(B)))
    out = np.stack([res.results[b]["out"] for b in range(B)], axis=0)
    return out.astype(np.float32)


if __name__ == "__main__":
    rng = np.random.default_rng(0)
    fake = {
        "input_ids": rng.integers(0, V, (B, S)),
        "emb": (rng.standard_normal((V, 2 * D)) * 0.02).astype(np.float32),
        "w_query": (rng.standard_normal((H, DH)) * 0.02).astype(np.float32),
        "b_query": np.zeros((H, DH), np.float32),
        "w_key": (rng.standard_normal((H, DH)) * 0.02).astype(np.float32),
        "b_key": np.zeros((H, DH), np.float32),
        "w_ctx": (rng.standard_normal((D, D)) * 0.02).astype(np.float32),
        "W_res": (rng.standard_normal((N, D)) * 0.02).astype(np.float32),
        "B_res": np.zeros((N, D), np.float32),
        "w_pr": (rng.standard_normal((D, N)) * 0.02).astype(np.float32),
        "w_pi": (rng.standard_normal((D, N)) * 0.02).astype(np.float32),
        "w_out": (rng.standard_normal((V, D)) * 0.02).astype(np.float32),
    }
    o = kernel(**fake)
    print("kernel ran:", o.shape, o.dtype, float(np.abs(o).mean()))
